# revision 1
# baseline (speedup 1.0000x reference)
"""Trainium2 Bass kernel for nn_CP_L3_sparse_outer.

Math (per batch row b):
    s2[b] = sum_d U2[d] * z[b, d]
    s3[b] = sum_d U3[d] * z[b, d]
    out[b, o] = (s2[b] * s3[b]) * sum_d (U1[d] * z[b, d]) * W[o, d] + bias[o]

Sharding: data-parallel over batch B=8192 across 8 NeuronCores
(B_loc = 1024 rows per core); W / U1 / U2 / U3 / bias replicated.

Per-core plan (f32 storage, main matmuls in float32r = 1 cyc/row at N=512):
  A. Load z row-tiles, stage through a DVE copy (collapses every PE
     transpose's waits onto the DVE semaphore), transpose 128x128 chunks on
     TensorE into resident ztbig = z.T [128 d_in, k(32) * 1024 b].
     Transposes write 4-chunk groups into one full PSUM bank so the bank WAR
     is dominated by the (newer) DVE data wait -> 1 sem wait per matmul
     (walrus allows only one on Matmult/DMACopy).
  B. s2/s3 via PE matmuls: psum[128 b, 2] += zT_chunk.T @ U23_chunk.
  C. c = s2*s3 -> per-tile PE transpose [128,1]->[1,128] -> ones[1,128]
     outer-product matmul -> cbcast [128, 1024] (c broadcast on partitions).
  D. zT = (zT * U1_per_partition) * cbcast in place (one DVE op per chunk),
     rounding to f32r on the write.
  E. Main matmul, output-transposed: per o-tile (32): psum [128 o, 512 b] x2
     accumulate over k with lhsT = W.T chunk (streamed), rhs = zT (resident);
     evict + bias via DVE tensor_scalar; transpose back on TensorE; batched
     SWDGE store to out[b, o].

Big/repeated DMAs go through SWDGE (gpsimd) whose ucode tolerates multiple
sem waits; HWDGE (sync) handles only the one-shot constant loads.
Host-side prep is layout-only: WT = W.T contiguous, U23 = stack(U2, U3).
"""

import os
import sys

import numpy as np

if "/opt/trn_rl_repo" not in sys.path:
    sys.path.insert(0, "/opt/trn_rl_repo")

import concourse.bass as bass
from concourse import bacc
import concourse.mybir as mybir
import concourse.tile as tile
from concourse.masks import make_identity

P = 128
D = 4096
O = 4096
B = 8192
NCORES = 8
BLOC = B // NCORES          # 1024 batch rows per core
KC = D // P                 # 32 contraction chunks
BT = BLOC // P              # 8 batch tiles of 128
OT = O // P                 # 32 output tiles of 128
NH = BLOC // 512            # 2 rhs halves of 512
QW = 1024                   # z row-segment width for phase A staging
NQ = D // QW                # 4 segments per batch tile
F32 = mybir.dt.float32
F32R = mybir.dt.float32r
MULT = mybir.AluOpType.mult


def build_nc() -> bass.Bass:
    nc = bacc.Bacc(trn_type="TRN2")

    z_d = nc.dram_tensor("z", [BLOC, D], F32, kind="ExternalInput")
    wt_d = nc.dram_tensor("wt", [D, O], F32R, kind="ExternalInput")
    u1_d = nc.dram_tensor("u1", [D], F32, kind="ExternalInput")
    u23_d = nc.dram_tensor("u23", [D, 2], F32, kind="ExternalInput")
    bias_d = nc.dram_tensor("bias", [O], F32, kind="ExternalInput")
    out_d = nc.dram_tensor("out", [BLOC, O], F32, kind="ExternalOutput")

    with tile.TileContext(nc) as tc:
        with (
            tc.tile_pool(name="const", bufs=1) as const,
            tc.tile_pool(name="ztp", bufs=1) as ztp,
            tc.tile_pool(name="znat", bufs=2) as znatp,
            tc.tile_pool(name="wslab", bufs=3) as wslabp,
            tc.tile_pool(name="outT", bufs=2) as outTp,
            tc.tile_pool(name="onat", bufs=2) as onatp,
            tc.tile_pool(name="pmain", bufs=4, space="PSUM") as pmain,
            tc.tile_pool(name="ptr", bufs=2, space="PSUM") as ptr,
            tc.tile_pool(name="pmisc", bufs=2, space="PSUM") as pmisc,
        ):
            # ---- constants (one-shot HWDGE loads) ----
            identity = const.tile([P, P], F32)
            make_identity(nc, identity)
            identity_r = const.tile([P, P], F32R)
            nc.vector.tensor_copy(identity_r[:], identity[:])
            ones1 = const.tile([1, P], F32)
            nc.vector.memset(ones1[:], 1.0)
            u1sb = const.tile([P, KC], F32)
            nc.sync.dma_start(u1sb[:], u1_d[:].rearrange("(k p) -> p k", p=P))
            u23raw = const.tile([P, KC, 2], F32)
            nc.sync.dma_start(u23raw[:], u23_d[:].rearrange("(k p) u -> p k u", p=P))
            u23sb = const.tile([P, KC, 2], F32R)
            nc.vector.tensor_copy(u23sb[:], u23raw[:])
            biassb = const.tile([P, OT], F32)
            nc.sync.dma_start(biassb[:], bias_d[:].rearrange("(a p) -> p a", p=P))
            t2row = const.tile([1, BLOC], F32)
            t3row = const.tile([1, BLOC], F32)
            crow = const.tile([1, BLOC], F32)
            cbcast = const.tile([P, BLOC], F32)

            # warm-up transpose (absorbs identity readiness once)
            ptw = ptr.tile([P, 512], F32R, name="pt", tag="pt")
            nc.tensor.transpose(ptw[:, 0:P], identity_r[:], identity_r[:])

            # zT resident: [128 d_in, k * BLOC + b]
            ztbig = ztp.tile([P, KC * BLOC], F32R)

            # ---- phase A: transpose z into ztbig (PE f32r + ACT copyback) ----
            for bt in range(BT):
                for q in range(NQ):
                    znat = znatp.tile([P, QW], F32R, name="znat")
                    nc.gpsimd.dma_start(
                        znat[:],
                        z_d[:][bt * P : (bt + 1) * P, q * QW : (q + 1) * QW],
                    )
                    for g in range(QW // 512):
                        pt = ptr.tile([P, 512], F32R, name="pt", tag="pt")
                        for i in range(4):
                            nc.tensor.transpose(
                                pt[:, i * P : (i + 1) * P],
                                znat[:, (g * 4 + i) * P : (g * 4 + i + 1) * P],
                                identity_r[:],
                            )
                        k0 = q * (QW // P) + g * 4
                        zt3 = ztbig[:].rearrange("p (k r) -> p k r", r=BLOC)
                        nc.scalar.activation(
                            zt3[:, k0 : k0 + 4, bt * P : (bt + 1) * P],
                            pt[:].rearrange("p (k r) -> p k r", r=P),
                            mybir.ActivationFunctionType.Copy,
                        )

            # ---- phase B: s2/s3 on PE, u23 stationary (2-col weight load),
            # output arrives transposed as rows [2, 512] ----
            for h in range(NH):
                for u, trow in enumerate([t2row, t3row]):
                    ps = pmisc.tile([1, 512], F32, name="ps23", tag="pmisc")
                    for k in range(KC):
                        nc.tensor.matmul(
                            ps[:],
                            u23sb[:, k, u : u + 1],
                            ztbig[
                                :, k * BLOC + h * 512 : k * BLOC + (h + 1) * 512
                            ],
                            start=(k == 0),
                            stop=(k == KC - 1),
                        )
                    nc.vector.tensor_copy(
                        trow[0:1, h * 512 : (h + 1) * 512], ps[0:1, :]
                    )

            # ---- phase C: c = s2*s3 -> broadcast across partitions ----
            nc.vector.tensor_mul(crow[0:1, :], t2row[0:1, :], t3row[0:1, :])
            for h in range(NH):
                pb = pmisc.tile([P, 512], F32, name="pb", tag="pmisc")
                nc.tensor.matmul(
                    pb[:], ones1[:],
                    crow[0:1, h * 512 : (h + 1) * 512],
                    start=True, stop=True,
                )
                nc.vector.tensor_copy(cbcast[:, h * 512 : (h + 1) * 512], pb[:])

            # ---- phase D: zT = (zT * U1) * c in place (rounds to f32r) ----
            for k in range(KC):
                sl = slice(k * BLOC, (k + 1) * BLOC)
                nc.vector.scalar_tensor_tensor(
                    ztbig[:, sl],
                    ztbig[:, sl],
                    u1sb[:, k : k + 1],
                    cbcast[:],
                    MULT,
                    MULT,
                )

            # ---- phase E: main matmul (float32r), evict, transpose out ----
            wt_view = wt_d[:].rearrange("(k p) o -> p k o", p=P)
            KH = KC // 2
            for ot in range(OT):
                wslabs = []
                for half in range(2):
                    ws = wslabp.tile([P, KH, P], F32R, name="wslab")
                    nc.gpsimd.dma_start(
                        ws[:],
                        wt_view[
                            :, half * KH : (half + 1) * KH, ot * P : (ot + 1) * P
                        ],
                    )
                    wslabs.append(ws)
                psums = [
                    pmain.tile([P, 512], F32, name=f"pm{h}", tag="pmain")
                    for h in range(NH)
                ]
                for k in range(KC):
                    lhs = wslabs[k // KH][:, k % KH, :]
                    for h in range(NH):
                        nc.tensor.matmul(
                            psums[h][:],
                            lhs,
                            ztbig[
                                :, k * BLOC + h * 512 : k * BLOC + (h + 1) * 512
                            ],
                            start=(k == 0),
                            stop=(k == KC - 1),
                        )
                outT = outTp.tile([P, BLOC], F32, name="outT")
                for h in range(NH):
                    nc.vector.tensor_scalar_add(
                        outT[:, h * 512 : (h + 1) * 512],
                        psums[h][:],
                        biassb[:, ot : ot + 1],
                    )
                onat = onatp.tile([P, BLOC], F32, name="onat")
                for g in range(BT // 4):
                    po = ptr.tile([P, 512], F32, name="pt", tag="pt")
                    for i in range(4):
                        bt = g * 4 + i
                        nc.tensor.transpose(
                            po[:, i * P : (i + 1) * P],
                            outT[:, bt * P : (bt + 1) * P],
                            identity[:],
                        )
                    nc.vector.tensor_copy(
                        onat[:, g * 512 : (g + 1) * 512], po[:]
                    )
                nc.gpsimd.dma_start(
                    out_d[:]
                    .rearrange("(t p) o -> p t o", p=P)[
                        :, :, ot * P : (ot + 1) * P
                    ],
                    onat[:].rearrange("p (t o) -> p t o", o=P),
                )

    nc.finalize()
    return nc


_NC_CACHE = {}


def get_nc() -> bass.Bass:
    if "nc" not in _NC_CACHE:
        _NC_CACHE["nc"] = build_nc()
    return _NC_CACHE["nc"]


def kernel(z, U1, U2, U3, W, b):
    from concourse.bass_utils import run_bass_kernel_spmd

    z = np.ascontiguousarray(np.asarray(z, dtype=np.float32)).reshape(B, D)
    U1 = np.asarray(U1, dtype=np.float32)
    U2 = np.asarray(U2, dtype=np.float32)
    U3 = np.asarray(U3, dtype=np.float32)
    W = np.asarray(W, dtype=np.float32)
    bias = np.asarray(b, dtype=np.float32)

    wt = np.ascontiguousarray(W.T)                      # [D, O], layout only
    u23 = np.ascontiguousarray(np.stack([U2, U3], 1))   # [D, 2]

    nc = get_nc()
    in_maps = [
        {
            "z": z[c * BLOC : (c + 1) * BLOC],
            "wt": wt,
            "u1": U1,
            "u23": u23,
            "bias": bias,
        }
        for c in range(NCORES)
    ]
    res = run_bass_kernel_spmd(
        nc,
        in_maps,
        core_ids=list(range(NCORES)),
        trace=bool(int(os.environ.get("KERNEL_TRACE", "0"))),
    )
    if res.exec_time_ns is not None:
        print(f"HW exec time: {res.exec_time_ns} ns", file=sys.stderr)
    kernel.last_results = res
    return np.concatenate([res.results[c]["out"] for c in range(NCORES)], axis=0)



# revision 8
# speedup vs baseline: 1.2106x; 1.2106x over previous
"""Trainium2 Bass kernel for nn_CP_L3_sparse_outer (v2, bf16).

Math (per batch row b):
    s2[b] = sum_d U2[d] * z[b, d]
    s3[b] = sum_d U3[d] * z[b, d]
    out[b, o] = (s2[b] * s3[b]) * sum_d (U1[d] * z[b, d]) * W[o, d] + bias[o]

Sharding: data-parallel over batch B=8192 across 8 NeuronCores
(B_loc = 1024 rows per core); W / U1 / U2 / U3 / bias replicated.

v2 design (vs v1 f32r): everything bf16 (measured rel-err 0.29% vs the
2e-2 gate), and the main matmul is flipped so psum is output-natural:

  A. z arrives bf16 [128 rows, 4096]; PE transposes (bf16 = 1 cyc/row)
     4-chunk groups into one PSUM bank; ACT copies into resident
     ztbig = z.T [128 d, k(32) * 1024 b], raw (no scaling).
  B. Per batch-tile s2/s3 on PE: psum[2, 128] += u23[128,2].T @ ztRAW,
     interleaved right after each tile's transposes.
  C. c = s2*s3 (DVE) -> 8 one-column micro-matmuls -> ccol [128 b, 8 bt]
     (c indexed by psum partition at eviction time).
  D. ztbig *= U1 per (k, bt) chunk in place on DVE (per-partition scalar,
     d on partitions), pipelined per tile behind B.
  E. Main matmul output-natural: for each o-chunk (8 x 512): stream
     wt slab [128 d, 32 k, 512 o] (double-buffered SWDGE), for each bt:
     psum[128 b, 512 o] += zt[k, bt].T(stationary) @ wt[k, oc](moving)
     over 32 k; evict with ONE DVE op: (psum * ccol) + biasb; one batched
     out DMA per o-chunk. No output transposes at all.

bias[o] lives on the free dim at eviction, so it is broadcast across
partitions once via ones-outer-product matmuls into biasb [128, 4096].
Host prep is dtype/layout only: bf16 casts + W.T contiguous.
"""

import os
import sys

import numpy as np

if "/opt/trn_rl_repo" not in sys.path:
    sys.path.insert(0, "/opt/trn_rl_repo")

import concourse.bass as bass
from concourse import bacc
import concourse.mybir as mybir
import concourse.tile as tile
from concourse.masks import make_identity

P = 128
D = 4096
O = 4096
B = 8192
NCORES = 8
BLOC = B // NCORES          # 1024 batch rows per core
KC = D // P                 # 32 contraction chunks
BT = BLOC // P              # 8 batch tiles of 128
OC = O // 512               # 8 output chunks of 512
F32 = mybir.dt.float32
BF16 = mybir.dt.bfloat16
MULT = mybir.AluOpType.mult
ADD = mybir.AluOpType.add
COPY = mybir.ActivationFunctionType.Copy


def build_nc() -> bass.Bass:
    nc = bacc.Bacc(trn_type="TRN2")

    z_d = nc.dram_tensor("z", [BLOC, D], BF16, kind="ExternalInput")
    wt_d = nc.dram_tensor("wt", [D, O], BF16, kind="ExternalInput")
    u1_d = nc.dram_tensor("u1", [D], F32, kind="ExternalInput")
    u23_d = nc.dram_tensor("u23", [D, 2], BF16, kind="ExternalInput")
    bias_d = nc.dram_tensor("bias", [O], BF16, kind="ExternalInput")
    out_d = nc.dram_tensor("out", [BLOC, O], F32, kind="ExternalOutput")

    zview = z_d[:].rearrange("(t p) d -> p t d", p=P)          # [128, 8, 4096]
    wview = wt_d[:].rearrange("(k p) o -> p k o", p=P)         # [128, 32, 4096]
    oview = out_d[:].rearrange("(t p) o -> p t o", p=P)        # [128, 8, 4096]

    with tile.TileContext(nc) as tc:
        with (
            tc.tile_pool(name="const", bufs=1) as const,
            tc.tile_pool(name="ztp", bufs=1) as ztp,
            tc.tile_pool(name="znat", bufs=2) as znatp,
            tc.tile_pool(name="wslab", bufs=2) as wslabp,
            tc.tile_pool(name="onat", bufs=2) as onatp,
            tc.tile_pool(name="pmain", bufs=3, space="PSUM") as pmain,
            tc.tile_pool(name="ptr", bufs=2, space="PSUM") as ptr,
            tc.tile_pool(name="pmisc", bufs=2, space="PSUM") as pmisc,
        ):
            # ---- constants (one-shot HWDGE loads) ----
            identity = const.tile([P, P], F32)
            make_identity(nc, identity)
            identity_b = const.tile([P, P], BF16)
            nc.vector.tensor_copy(identity_b[:], identity[:])
            ones1 = const.tile([1, P], BF16)
            nc.vector.memset(ones1[:], 1.0)
            onef = const.tile([1, 1], F32)
            nc.vector.memset(onef[:], 1.0)
            u1sb = const.tile([P, KC], F32)
            nc.sync.dma_start(u1sb[:], u1_d[:].rearrange("(k p) -> p k", p=P))
            u23sb = const.tile([P, KC, 2], BF16)
            nc.sync.dma_start(u23sb[:], u23_d[:].rearrange("(k p) u -> p k u", p=P))
            # s2/s3 psum rows must land on 32-aligned partitions: put U2 in
            # stationary column 0 and U3 in column 32 of a zero-padded lhsT.
            u23pad = const.tile([P, KC, 64], BF16)
            nc.vector.memset(u23pad[:], 0.0)
            nc.vector.tensor_copy(u23pad[:, :, 0:1], u23sb[:, :, 0:1])
            nc.vector.tensor_copy(u23pad[:, :, 32:33], u23sb[:, :, 1:2])
            biasrow = znatp.tile([1, O], BF16, name="znat")
            nc.sync.dma_start(biasrow[:], bias_d[:].rearrange("(a o) -> a o", a=1))
            biasb = const.tile([P, O], BF16)
            t2row = const.tile([1, BLOC], F32)
            t3row = const.tile([1, BLOC], F32)
            ccol = const.tile([P, BT], F32)

            # warm-up transpose (absorbs identity readiness once)
            ptw = ptr.tile([P, 512], BF16, name="pt", tag="pt")
            nc.tensor.transpose(ptw[:, 0:P], identity_b[:], identity_b[:])

            # bias broadcast across partitions: biasb[p, o] = bias[o]
            for oc in range(OC):
                pb = pmisc.tile([P, 512], F32, name="pb", tag="pmisc")
                nc.tensor.matmul(
                    pb[:], ones1[:], biasrow[0:1, oc * 512 : (oc + 1) * 512],
                    start=True, stop=True,
                )
                nc.scalar.activation(biasb[:, oc * 512 : (oc + 1) * 512], pb[:], COPY)

            # zT resident: [128 d_in, k * BLOC + b]
            ztbig = ztp.tile([P, KC * BLOC], BF16)
            zt3 = ztbig[:].rearrange("p (k r) -> p k r", r=BLOC)

            # ---- phases A/B/D interleaved per batch tile ----
            for bt in range(BT):
                znat = znatp.tile([P, D], BF16, name="znat")
                nc.gpsimd.dma_start(znat[:], zview[:, bt, :])
                # A: transpose 32 chunks in groups of 4 -> one PSUM bank each
                for g in range(KC // 4):
                    pt = ptr.tile([P, 512], BF16, name="pt", tag="pt")
                    for i in range(4):
                        nc.tensor.transpose(
                            pt[:, i * P : (i + 1) * P],
                            znat[:, (g * 4 + i) * P : (g * 4 + i + 1) * P],
                            identity_b[:],
                        )
                    nc.scalar.activation(
                        zt3[:, g * 4 : g * 4 + 4, bt * P : (bt + 1) * P],
                        pt[:].rearrange("p (k r) -> p k r", r=P),
                        COPY,
                    )
                # B: s2/s3 for this tile from RAW zt
                ps23 = pmisc.tile([64, P], F32, name="ps23", tag="pmisc")
                for k in range(KC):
                    nc.tensor.matmul(
                        ps23[:],
                        u23pad[:, k, :],
                        zt3[:, k, bt * P : (bt + 1) * P],
                        start=(k == 0),
                        stop=(k == KC - 1),
                    )
                nc.vector.tensor_copy(t2row[0:1, bt * P : (bt + 1) * P], ps23[0:1, :])
                nc.vector.tensor_copy(t3row[0:1, bt * P : (bt + 1) * P], ps23[32:33, :])
                # D: fold U1 into zt in place (per-partition scalar per chunk)
                for k in range(KC):
                    nc.vector.tensor_scalar_mul(
                        zt3[:, k, bt * P : (bt + 1) * P],
                        zt3[:, k, bt * P : (bt + 1) * P],
                        u1sb[:, k : k + 1],
                    )

            # ---- phase C: c = s2*s3 (in place on t2row) -> ccol [128 b, bt] ----
            nc.vector.tensor_mul(t2row[0:1, :], t2row[0:1, :], t3row[0:1, :])
            pc = pmisc.tile([P, BT], F32, name="pc", tag="pmisc")
            for g in range(BT):
                nc.tensor.matmul(
                    pc[:, g : g + 1],
                    t2row[0:1, g * P : (g + 1) * P],
                    onef[0:1, 0:1],
                    start=True, stop=True,
                )
            nc.vector.tensor_copy(ccol[:], pc[:])

            # ---- phase E: main matmul, output-natural psum [b, o] ----
            for oc in range(OC):
                ws = wslabp.tile([P, KC, 512], BF16, name="wslab")
                nc.gpsimd.dma_start(ws[:], wview[:, :, oc * 512 : (oc + 1) * 512])
                onat = onatp.tile([P, BT, 512], F32, name="onat")
                for bt in range(BT):
                    pm = pmain.tile([P, 512], F32, name="pm", tag="pmain")
                    for k in range(KC):
                        nc.tensor.matmul(
                            pm[:],
                            zt3[:, k, bt * P : (bt + 1) * P],
                            ws[:, k, :],
                            start=(k == 0),
                            stop=(k == KC - 1),
                        )
                    nc.vector.scalar_tensor_tensor(
                        onat[:, bt, :],
                        pm[:],
                        ccol[:, bt : bt + 1],
                        biasb[:, oc * 512 : (oc + 1) * 512],
                        MULT,
                        ADD,
                    )
                nc.gpsimd.dma_start(
                    oview[:, :, oc * 512 : (oc + 1) * 512], onat[:]
                )

    nc.finalize()
    return nc


_NC_CACHE = {}


def get_nc() -> bass.Bass:
    if "nc" not in _NC_CACHE:
        _NC_CACHE["nc"] = build_nc()
    return _NC_CACHE["nc"]


def kernel(z, U1, U2, U3, W, b):
    import ml_dtypes
    from concourse.bass_utils import run_bass_kernel_spmd

    bf = ml_dtypes.bfloat16
    z = np.ascontiguousarray(np.asarray(z, dtype=np.float32)).reshape(B, D)
    zq = z.astype(bf)
    U1 = np.asarray(U1, dtype=np.float32)
    wt = np.ascontiguousarray(np.asarray(W, dtype=np.float32).T).astype(bf)
    u23 = np.ascontiguousarray(
        np.stack([np.asarray(U2, dtype=np.float32),
                  np.asarray(U3, dtype=np.float32)], 1)
    ).astype(bf)
    bias = np.asarray(b, dtype=np.float32).astype(bf)

    nc = get_nc()
    in_maps = [
        {
            "z": zq[c * BLOC : (c + 1) * BLOC],
            "wt": wt,
            "u1": U1,
            "u23": u23,
            "bias": bias,
        }
        for c in range(NCORES)
    ]
    res = run_bass_kernel_spmd(
        nc,
        in_maps,
        core_ids=list(range(NCORES)),
        trace=bool(int(os.environ.get("KERNEL_TRACE", "0"))),
    )
    if res.exec_time_ns is not None:
        print(f"HW exec time: {res.exec_time_ns} ns", file=sys.stderr)
    kernel.last_results = res
    return np.concatenate([res.results[c]["out"] for c in range(NCORES)], axis=0)
